# revision 47
# baseline (speedup 1.0000x reference)
"""Trainium2 Bass kernel for a GPT-style transformer block (B=2, T=2048, C=768,
NH=12, HD=64, DFF=3072), distributed over 8 NeuronCores.

Sharding: token-data-parallel with zigzag strip assignment, zero collectives.
  - cores 0-3 process batch 0, cores 4-7 batch 1.
  - within a batch, rank r owns token strips r and 7-r (strips of 256 tokens).
  - each core redundantly computes K/V for tokens [0, 256*(8-r)) (its causal
    prefix), so no cross-core communication is needed at all.

All GEMMs run in bf16 (weights cast on host; activations cast on the fly by
the producing engine), accumulating in fp32 PSUM.  The residual/LayerNorm path
stays fp32.  LN affine params are folded into the adjacent weights on the host.
The K bias is dropped entirely (a per-query-column constant in the logits,
which cancels in softmax).  The V bias is folded into the per-head epilogue
after softmax normalization.

Attention uses exp-without-max softmax (safe for this distribution) computed in
a transposed layout so the softmax denominator comes from an appended ones row.
QK^T matmuls for the even/odd head of a pair run concurrently in the top/bottom
halves of the PE array (row tiling via tile_position).

Emission is software-pipelined: LN1+transpose waves interleave with the K/V
GEMM stream, and attention(ph=0) interleaves with the ph=1 K/V/Q GEMMs, under
an exact 8-bank PSUM budget.
"""

import sys
import types
import functools

sys.path.insert(0, "/opt/trn_rl_repo")

# ---- antenv.axon_hooks shim (missing module in this image) -----------------
if "antenv.axon_hooks" not in sys.modules:
    _hooks = types.ModuleType("antenv.axon_hooks")
    _hooks._hook = None
    _hooks.set_axon_ntff_profile_hook = lambda h: setattr(_hooks, "_hook", h)
    _hooks.get_axon_ntff_profile_hook = lambda: _hooks._hook
    sys.modules["antenv.axon_hooks"] = _hooks
    try:
        import antenv

        antenv.axon_hooks = _hooks
    except ImportError:
        pass

import numpy as np
import jax
import ml_dtypes

import concourse.bass as bass
import concourse.mybir as mybir
import concourse.tile as tile
from concourse import bacc
from concourse.bass2jax import (
    _bass_exec_p,
    install_neuronx_cc_hook,
    partition_id_tensor,
)
from concourse.masks import make_identity

B, T, C = 2, 2048, 768
NH, HD, DFF = 12, 64, 64 * 48  # DFF = 3072
F32 = mybir.dt.float32
BF16 = mybir.dt.bfloat16
EPS = 1e-5
AF = mybir.ActivationFunctionType


# ---------------------------------------------------------------------------
# Per-rank program builder
# ---------------------------------------------------------------------------
def build_rank_program(r: int):
    """Program for rank r (strips r and 7-r of one batch element)."""
    nc = bacc.Bacc("TRN2", target_bir_lowering=False, debug=False, num_devices=1)

    x_in = nc.declare_dram_parameter("x", [T, C], F32, isOutput=False)
    wq_in = nc.declare_dram_parameter("wq", [C, C], BF16, isOutput=False)
    wk_in = nc.declare_dram_parameter("wk", [C, C], BF16, isOutput=False)
    wv_in = nc.declare_dram_parameter("wv", [C, C], BF16, isOutput=False)
    bq_in = nc.declare_dram_parameter("bq", [C], F32, isOutput=False)
    bv_in = nc.declare_dram_parameter("bv", [C], F32, isOutput=False)
    wcp_in = nc.declare_dram_parameter("wcp", [C, C], BF16, isOutput=False)
    bcp_in = nc.declare_dram_parameter("bcp", [C], F32, isOutput=False)
    wfc_in = nc.declare_dram_parameter("wfc", [C, DFF], BF16, isOutput=False)
    bfc_in = nc.declare_dram_parameter("bfc", [DFF], F32, isOutput=False)
    wpj_in = nc.declare_dram_parameter("wpj", [DFF, C], BF16, isOutput=False)
    bpj_in = nc.declare_dram_parameter("bpj", [C], F32, isOutput=False)
    out_dram = nc.declare_dram_parameter("out", [512, C], F32, isOutput=True)

    ins = dict(x=x_in, wq=wq_in, wk=wk_in, wv=wv_in, bq=bq_in, bv=bv_in,
               wcp=wcp_in, bcp=bcp_in, wfc=wfc_in, bfc=bfc_in,
               wpj=wpj_in, bpj=bpj_in, out=out_dram)
    with tile.TileContext(nc) as tc:
        _build_body(nc, tc, r, ins)
    nc.compile()
    return nc


def _build_body(nc, tc, r, ins):
    from contextlib import ExitStack

    sA, sB = r, 7 - r
    NTK = 2 * (8 - r)          # kt tiles of 128 in the causal prefix
    T_kv = NTK * 128
    NB = 8 - r                 # 256-token waves in the prefix

    x_in = ins["x"]

    with ExitStack() as ctx:
        # ------- pools for persistent tensors -------
        const = ctx.enter_context(tc.tile_pool(name="const", bufs=1))
        acts = ctx.enter_context(tc.tile_pool(name="acts", bufs=1))
        yT_sb = acts.tile([128, 6, 512], BF16)         # y cols x own q
        # fc weights resident for the whole MLP (DMA'd during attention)
        wfc_t = acts.tile([128, 6, DFF], BF16)

        s123 = ctx.enter_context(ExitStack())
        acts13 = s123.enter_context(tc.tile_pool(name="acts13", bufs=1))
        hT_sb = acts13.tile([128, 6, T_kv], BF16)      # ln1(x) transposed
        qT_sb = acts13.tile([128, 6, 512], BF16)       # head-pair rows x own q
        kv_pool = s123.enter_context(tc.tile_pool(name="kv", bufs=1))
        kT_sb = [kv_pool.tile([128, 3, T_kv], BF16, tag=f"kT{p}", name=f"kT{p}")
                 for p in range(2)]
        v_sb = [kv_pool.tile([128, NTK, 6, 65], BF16, tag=f"v{p}", name=f"v{p}")
                for p in range(2)]
        w_pool = s123.enter_context(tc.tile_pool(name="wkvq", bufs=1))
        wk_t, wv_t, wq_t = [], [], []
        for p in range(2):
            for wname, lst in (("wk", wk_t), ("wv", wv_t), ("wq", wq_t)):
                wt = w_pool.tile([128, 6, 384], BF16, tag=f"{wname}_{p}",
                                 name=f"{wname}{p}")
                lst.append(wt)

        def emit_w_dma(p):
            co = p * 384
            for src_in, lst in ((ins["wk"], wk_t), (ins["wv"], wv_t),
                                (ins["wq"], wq_t)):
                nc.sync.dma_start(
                    out=lst[p][:],
                    in_=src_in[:, co:co + 384].rearrange("(c k) n -> k c n", k=128))

        # K/V/Q matmul PSUM pool: outlives stage 1 (the ph=1 GEMMs interleave
        # with attention ph=0) but closes before attention ph=1
        mm_scope = ExitStack()
        mm_ps = mm_scope.enter_context(tc.tile_pool(name="mm_ps", bufs=2, space="PSUM"))

        # ------- stage-1-only pools (closed right after stage 1 to free
        # SBUF/PSUM for the attention pipeline + resident fc weights) -------
        s1_scope = ExitStack()
        ln_x = s1_scope.enter_context(tc.tile_pool(name="ln_x", bufs=3))
        ln_pool = s1_scope.enter_context(tc.tile_pool(name="ln", bufs=2))
        tp_ps = s1_scope.enter_context(tc.tile_pool(name="tp_ps", bufs=2, space="PSUM"))
        x_q = []

        def x_dma(b2):
            x2_t = ln_x.tile([128, 2, C], F32, tag="x", name=f"x2_{b2}")
            nc.sync.dma_start(
                out=x2_t[:],
                in_=x_in[b2 * 256:(b2 + 1) * 256, :].rearrange("(t p) c -> p t c", p=128))
            x_q.append(x2_t)

        for b2_ in range(min(3, NB)):
            x_dma(b2_)
        emit_w_dma(0)
        wcp_t = const.tile([128, 6, C], BF16)

        # ------- constants needed during stage 1 (keep the DVE queue clear
        # ahead of the first LN wave; everything else is set up later) -------
        id_f = const.tile([128, 128], F32)
        make_identity(nc, id_f[:])
        id_b = const.tile([128, 128], BF16)
        nc.vector.tensor_copy(id_b[:], id_f[:])
        eps_t = const.tile([128, 1], F32)
        nc.vector.memset(eps_t[:], EPS)
        # own-x rows for the stage-4 residual: copied out of the stage-1 wave
        # tiles on the (otherwise idle) GpSimd engine — no extra DMA
        x_own = [const.tile([128, C], F32, tag=f"xo{m}", name=f"xo{m}")
                 for m in range(4)]

        def save_x_own(b2, x2_t):
            for m, (sw, tt) in enumerate(((sA, 0), (sA, 1), (sB, 0), (sB, 1))):
                if sw == b2:
                    nc.gpsimd.tensor_copy(x_own[m][:], x2_t[:, tt, :])

        # per-partition bias tiles [128, 6] (column j = head-pair j)
        bq_sb = const.tile([128, 6], F32)
        bv_sb = const.tile([128, 6], F32)
        for src, dst in ((ins["bq"], bq_sb), (ins["bv"], bv_sb)):
            nc.sync.dma_start(out=dst[:], in_=src[:].rearrange("(j p) -> p j", p=128))

        def emit_late_consts():
            """Masks, broadcast biases, V ones-rows — needed from attention
            and stage 4 on; emitted after stage 1 so they don't delay it."""
            nc.vector.memset(mask_t[:], 1.0)
            for off in range(2):
                nc.gpsimd.affine_select(
                    out=mask_t[:, off, :],
                    in_=mask_t[:, off, :],
                    compare_op=mybir.AluOpType.is_ge,
                    fill=0.0,
                    base=-128 * off,
                    pattern=[[1, 256]],
                    channel_multiplier=-1,
                )
            nc.sync.dma_start(out=bfc_sb[:], in_=ins["bfc"][:].rearrange("(f p) -> p f", p=128))
            nc.sync.dma_start(out=brow_f[:, 0, :], in_=ins["bcp"][:][None, :])
            nc.sync.dma_start(out=brow_f[:, 1, :], in_=ins["bpj"][:][None, :])
            nc.gpsimd.partition_broadcast(bias_bc[:], brow_f[:])
            for p in range(2):
                for ti in range(NTK):
                    nc.vector.memset(v_sb[p][:, ti, :, 64], 1.0)

        # causal masks for the two in-strip kt chunk offsets: [128, 2, 256]
        mask_t = const.tile([128, 2, 256], BF16)
        bfc_sb = const.tile([128, 24], F32)
        brow_f = const.tile([1, 2, C], F32)
        bias_bc = const.tile([128, 2, C], F32)
        bcp_bc = bias_bc[:, 0, :]
        bpj_bc = bias_bc[:, 1, :]

        # =========== stage 1 + stage 2(ph0): LN1+transpose || K/V GEMM ======
        # Emission interleaves LN waves (DVE/ACT-heavy) with K/V matmuls
        # (PE-heavy) so no engine starves.

        def emit_ln_wave(b2):
            """LN1 for tokens [b2*256, (b2+1)*256); returns the two h tiles."""
            x2_t = x_q[b2]
            mv = ln_pool.tile([128, 2, 2], F32, tag="mv")
            for tt in range(2):
                xg = x2_t[:, tt, :].rearrange("p (g d) -> p g d", g=2)
                stats = ln_pool.tile([128, 2, 6], F32, tag="st")
                for g in range(2):
                    nc.vector.bn_stats(out=stats[:, g, :], in_=xg[:, g, :])
                nc.vector.bn_aggr(out=mv[:, tt, :], in_=stats[:])
            rstd = ln_pool.tile([128, 2], F32, tag="rstd")
            nc.scalar.activation(
                out=rstd[:], in_=mv[:, :, 1],
                func=AF.Sqrt, bias=eps_t[:], scale=1.0,
            )
            nc.vector.reciprocal(out=rstd[:], in_=rstd[:])
            nmu = ln_pool.tile([128, 2], F32, tag="nmu")
            nc.vector.tensor_tensor(
                out=nmu[:], in0=mv[:, :, 0], in1=rstd[:],
                op=mybir.AluOpType.mult,
            )
            nc.vector.tensor_scalar(
                out=nmu[:], in0=nmu[:],
                scalar1=-1.0, scalar2=None, op0=mybir.AluOpType.mult,
            )
            hs = []
            for tt in range(2):
                h_t = ln_pool.tile([128, C], BF16, tag=f"h{tt}", name=f"h{b2}_{tt}")
                nc.scalar.activation(
                    out=h_t[:], in_=x2_t[:, tt, :], func=AF.Identity,
                    bias=nmu[:, tt:tt + 1], scale=rstd[:, tt:tt + 1],
                )
                hs.append(h_t)
            return hs

        def emit_tp_wave(b2, hs):
            """Transpose wave b2's h tiles into hT."""
            tp = tp_ps.tile([128, 6, 2, 128], BF16, tag="tp")
            for c in range(6):
                for tt in range(2):
                    nc.tensor.transpose(tp[:, c, tt, :], hs[tt][:, c * 128:(c + 1) * 128], id_b[:])
            nc.vector.tensor_copy(
                hT_sb[:, :, b2 * 256:(b2 + 1) * 256],
                tp[:].rearrange("p c t n -> p c (t n)"),
            )

        def emit_k_block(p, tb, bw):
            """kT[p][:, :, tb:tb+bw] (bw <= 1024 for ph0, 512 for ph1).
            ph0 evicts on ACT (idle in the LN window); ph1 evicts on DVE
            (ACT is exp-bound then)."""
            for j in range(3):
                pk = mm_ps.tile([128, bw], F32, tag="pk",
                                padded_shape=[128, 512], name="pk")
                for c in range(6):
                    nc.tensor.matmul(
                        pk[:, 0:bw], wk_t[p][:, c, j * 128:(j + 1) * 128],
                        hT_sb[:, c, tb:tb + bw],
                        start=(c == 0), stop=(c == 5),
                    )
                if p == 0:
                    nc.scalar.copy(kT_sb[p][:, j, tb:tb + bw], pk[:, 0:bw])
                else:
                    nc.vector.tensor_copy(kT_sb[p][:, j, tb:tb + bw], pk[:, 0:bw])

        def emit_v_tile(p, ti):
            pv = mm_ps.tile([128, 384], F32, tag="pk", padded_shape=[128, 512],
                            name="pv")
            for c in range(6):
                nc.tensor.matmul(
                    pv[:], hT_sb[:, c, ti * 128:(ti + 1) * 128],
                    wv_t[p][:, c, :],
                    start=(c == 0), stop=(c == 5),
                )
            if p == 0:
                nc.scalar.copy(
                    v_sb[p][:, ti, :, 0:64],
                    pv[:].rearrange("p (h d) -> p h d", d=64),
                )
            else:
                nc.vector.tensor_copy(
                    v_sb[p][:, ti, :, 0:64],
                    pv[:].rearrange("p (h d) -> p h d", d=64),
                )

        def emit_q(p):
            """Q GEMM for own strips (both strips in one N=512 matmul)."""
            tbA, tbB = sA * 256, sB * 256
            for j in range(3):
                jj = 3 * p + j
                pq = mm_ps.tile([128, 512], F32, tag="pk",
                                padded_shape=[128, 512], name="pq")
                for c in range(6):
                    rhs = bass.AP(
                        tensor=hT_sb[:, c, :].tensor,
                        offset=hT_sb[:, c, tbA:tbA + 1].offset,
                        ap=[list(q) for q in hT_sb[:, c, :].ap[:1]]
                        + [[hT_sb[:, c, :].ap[-1][0] * (tbB - tbA), 2],
                           [hT_sb[:, c, :].ap[-1][0], 256]],
                    )
                    nc.tensor.matmul(
                        pq[:], wq_t[p][:, c, j * 128:(j + 1) * 128], rhs,
                        start=(c == 0), stop=(c == 5),
                    )
                nc.vector.tensor_scalar(
                    out=qT_sb[:, jj, :], in0=pq[:],
                    scalar1=bq_sb[:, jj:jj + 1], scalar2=None,
                    op0=mybir.AluOpType.add,
                )

        # interleave: LN(w) [DVE/ACT] -> K/V fill through (w-1)*256 [PE] ->
        # transpose wave w-1 [PE].  Transposes run one wave behind LN so the
        # PE never waits on the LN chain; K/V matmuls fill the gaps.
        kv_state = {"k": 0, "v": 0}

        def fill_kv0(ready):
            while (kv_state["k"] + 512 <= ready
                   or (ready == T_kv and kv_state["k"] < T_kv)):
                bw = min(512, T_kv - kv_state["k"])
                emit_k_block(0, kv_state["k"], bw)
                kv_state["k"] += bw
            while (kv_state["v"] + 1) * 128 <= ready:
                emit_v_tile(0, kv_state["v"])
                kv_state["v"] += 1

        pending_tp = None
        for b2 in range(NB):
            if b2 + 3 < NB:
                x_dma(b2 + 3)
            hs = emit_ln_wave(b2)
            save_x_own(b2, x_q[b2])
            if pending_tp is not None:
                fill_kv0(pending_tp[0] * 256)
                emit_tp_wave(*pending_tp)
            pending_tp = (b2, hs)
        fill_kv0((NB - 1) * 256)
        emit_tp_wave(*pending_tp)
        fill_kv0(T_kv)
        emit_q(0)
        s1_scope.close()  # free stage-1 SBUF/PSUM before attention
        emit_late_consts()

        # =================== stage 3: attention =============================
        # chunks < n_sh apply to both strips (N=512); rest strip-B only
        n_sh = 2 * (sA + 1)
        n_all = 2 * (sB + 1)

        def emit_attention(ph, att_ps, yt_ps, att_pool, nrm_pool, filler):
            """Attention for heads [6*ph, 6*ph+6), head pairs row-tiled.

            filler: list of zero-arg callables; one is popped and emitted
            after each head pair to interleave PE-dense work under the
            ACT-bound softmax stream.
            """
            for pr in range(3):
                jj = 3 * ph + pr          # head-pair index
                h0 = 6 * ph + 2 * pr      # even head
                j3 = pr                   # kT column for this pair
                yt = [yt_ps.tile([65, 512], F32, tag="yt", name=f"yt{eo_}")
                      for eo_ in range(2)]
                pending = None

                def issue_av(pend):
                    for kc, at_sl, qs, ww, eo in pend:
                        nc.tensor.matmul(
                            yt[eo][0:65, qs:qs + ww],
                            v_sb[ph][:, kc, 2 * j3 + eo, 0:65],
                            at_sl[:, 0:ww],
                            start=(kc == 0), stop=(kc == n_all - 1),
                            skip_group_check=True,
                        )

                for kp in range(n_all // 2):
                    kc0 = 2 * kp
                    shared = kc0 < n_sh
                    ww = 512 if shared else 256
                    qs = 0 if shared else 256
                    new_pend = []
                    if shared:
                        # two 2-bank tiles (one per head), exp each
                        pa = [att_ps.tile([128, 2, 512], F32, tag="pa",
                                          name=f"pa{eo_}")
                              for eo_ in range(2)]
                        for u in range(2):
                            for eo in range(2):
                                nc.tensor.matmul(
                                    pa[eo][:, u, :],
                                    kT_sb[ph][64 * eo:64 * eo + 64, j3,
                                              (kc0 + u) * 128:(kc0 + u + 1) * 128],
                                    qT_sb[64 * eo:64 * eo + 64, jj, :],
                                    start=True, stop=True,
                                )
                        at = att_pool.tile([128, 2, 2, 512], BF16, tag="at",
                                           bufs=2)
                        for eo in range(2):
                            nc.scalar.activation(
                                out=at[:, eo, :, :], in_=pa[eo][:],
                                func=AF.Exp,
                            )
                        for u in range(2):
                            for eo in range(2):
                                new_pend.append(
                                    (kc0 + u, at[:, eo, u, :], 0, 512, eo))
                    else:
                        pa = att_ps.tile([128, 2, 2, 256], F32, tag="pa")
                        for u in range(2):
                            for eo in range(2):
                                nc.tensor.matmul(
                                    pa[:, eo, u, :],
                                    kT_sb[ph][64 * eo:64 * eo + 64, j3,
                                              (kc0 + u) * 128:(kc0 + u + 1) * 128],
                                    qT_sb[64 * eo:64 * eo + 64, jj, 256:512],
                                    start=True, stop=True,
                                )
                        at = att_pool.tile([128, 2, 2, 256], BF16, tag="at2")
                        nc.scalar.activation(out=at[:], in_=pa[:], func=AF.Exp)
                        for u in range(2):
                            for eo in range(2):
                                new_pend.append(
                                    (kc0 + u, at[:, eo, u, :], 256, 256, eo))
                    # in-strip causal masks
                    for u in range(2):
                        kc = kc0 + u
                        for eo in range(2):
                            at_u = new_pend[2 * u + eo][1]
                            if kc in (2 * sA, 2 * sA + 1):
                                nc.vector.tensor_mul(
                                    at_u[:, 0:256], at_u[:, 0:256],
                                    mask_t[:, kc - 2 * sA, :])
                            if kc in (2 * sB, 2 * sB + 1):
                                boff = 256 if shared else 0
                                nc.vector.tensor_mul(
                                    at_u[:, boff:boff + 256],
                                    at_u[:, boff:boff + 256],
                                    mask_t[:, kc - 2 * sB, :])
                    if pending is not None:
                        issue_av(pending)
                    pending = new_pend
                issue_av(pending)
                # ---- normalize + V-bias epilogue for both heads.  The yt
                # PSUM banks are evicted raw first so the next pair's AV
                # accumulation isn't gated on this (slow) normalize chain. ----
                for eo in range(2):
                    h = h0 + eo
                    po = 64 * eo
                    sume = nrm_pool.tile([1, 512], F32, tag="sume")
                    nc.vector.tensor_copy(sume[:], yt[eo][64:65, :])
                    yraw = nrm_pool.tile([64, 512], F32, tag="yraw",
                                         name=f"yraw{eo}")
                    nc.vector.tensor_copy(yraw[:], yt[eo][0:64, :])
                    bcast = nrm_pool.tile([64, 512], F32, tag="bcast",
                                          bufs=1)
                    nc.gpsimd.partition_broadcast(bcast[:], sume[:])
                    nc.vector.reciprocal_approx_fast(out=bcast[:], in_=bcast[:])
                    nc.vector.tensor_mul(
                        yT_sb[po:po + 64, jj, :], yraw[:], bcast[:],
                    )
                    nc.vector.tensor_scalar(
                        out=yT_sb[po:po + 64, jj, :],
                        in0=yT_sb[po:po + 64, jj, :],
                        scalar1=bv_sb[po:po + 64, jj:jj + 1], scalar2=None,
                        op0=mybir.AluOpType.add,
                    )
                if filler:
                    filler.pop(0)()

        # ---- attention ph0 interleaved with stage 2(ph1) fillers ----
        emit_w_dma(1)
        # resident MLP weights: DMA'd during the attention window (DMA
        # otherwise idle); tiles live in ctx-level pools so the transfers
        # aren't gated on attention-era SBUF reuse
        nc.sync.dma_start(out=wcp_t[:], in_=ins["wcp"][:].rearrange("(j k) n -> k j n", k=128))
        nc.sync.dma_start(out=wfc_t[:], in_=ins["wfc"][:].rearrange("(c k) n -> k c n", k=128))

        def fill_k1():
            tb = 0
            while tb < T_kv:
                bw = min(512, T_kv - tb)
                emit_k_block(1, tb, bw)
                tb += bw

        def fill_v1():
            for ti in range(NTK):
                emit_v_tile(1, ti)

        att0_scope = ExitStack()
        att_ps = att0_scope.enter_context(tc.tile_pool(name="att_ps", bufs=2, space="PSUM"))
        yt_ps = att0_scope.enter_context(tc.tile_pool(name="yt_ps", bufs=2, space="PSUM"))
        att_pool = att0_scope.enter_context(tc.tile_pool(name="att", bufs=2))
        nrm_pool = att0_scope.enter_context(tc.tile_pool(name="nrm", bufs=2))
        emit_attention(0, att_ps, yt_ps, att_pool, nrm_pool,
                       [fill_k1, fill_v1, lambda: emit_q(1)])
        att0_scope.close()
        mm_scope.close()  # ph1 GEMMs done; their PSUM buys attention ph1 a
        # deeper QK->exp->AV pipeline (pa bufs=3)
        att1_scope = ExitStack()
        att_ps = att1_scope.enter_context(tc.tile_pool(name="att1_ps", bufs=3, space="PSUM"))
        yt_ps = att1_scope.enter_context(tc.tile_pool(name="yt1_ps", bufs=2, space="PSUM"))
        att_pool = att1_scope.enter_context(tc.tile_pool(name="att1", bufs=3))
        nrm_pool = att1_scope.enter_context(tc.tile_pool(name="nrm1", bufs=2))
        emit_attention(1, att_ps, yt_ps, att_pool, nrm_pool, [])
        att1_scope.close()
        s123.close()  # free hT/qT/kT/v/weights SBUF before the MLP stages
        # preload the sqrt table set while stage-4 residual adds run
        sq_warm = const.tile([128, 1], F32)
        nc.scalar.activation(out=sq_warm[:], in_=eps_t[:], func=AF.Sqrt,
                             bias=0.0, scale=1.0)

        # =================== stages 4-6: c_proj, MLP ========================
        with ExitStack() as s46:
            act46 = s46.enter_context(tc.tile_pool(name="act46", bufs=1))
            ln2_pool = s46.enter_context(tc.tile_pool(name="ln2", bufs=2))
            out_pool = s46.enter_context(tc.tile_pool(name="outp", bufs=3))

            x1_sb = act46.tile([128, 4, C], F32)
            h2T_sb = act46.tile([128, 6, 512], BF16)
            gT_sb = act46.tile([128, 24, 512], BF16)
            wpj_t = act46.tile([128, 24, C], BF16)
            nc.sync.dma_start(out=wpj_t[:], in_=ins["wpj"][:].rearrange("(f k) n -> k f n", k=128))

            # ---- stage 4: c_proj (all blocks) then residual+LN2+transpose.
            # The LN2 chain is two-pass software-pipelined: stats for all m
            # first, then apply+transpose, so the PE never waits on a full
            # per-m LN round trip. ----
            s4 = ExitStack()
            tp2_ps = s4.enter_context(tc.tile_pool(name="tp2_ps", bufs=2, space="PSUM"))
            cp_ps = s4.enter_context(tc.tile_pool(name="cp_ps", bufs=3, space="PSUM"))
            pp = []
            for m in range(4):
                for half in range(2):
                    pp_i = cp_ps.tile([128, 384], F32, tag=f"cp{half}",
                                      name=f"cp{m}_{half}")
                    pp.append(pp_i)
                    for j in range(6):
                        nc.tensor.matmul(
                            pp_i[:],
                            yT_sb[:, j, m * 128:(m + 1) * 128],
                            wcp_t[:, j, half * 384:(half + 1) * 384],
                            start=(j == 0), stop=(j == 5),
                        )
            mv_all = ln2_pool.tile([128, 4, 2], F32, tag="mv2", bufs=1)
            for m in range(4):
                nc.vector.tensor_add(x_own[m][:], x_own[m][:], bcp_bc[:])
                for half in range(2):
                    nc.vector.tensor_add(
                        x1_sb[:, m, half * 384:(half + 1) * 384],
                        pp[m * 2 + half][:],
                        x_own[m][:, half * 384:(half + 1) * 384],
                    )
                x1g = x1_sb[:, m, :].rearrange("p (g d) -> p g d", g=2)
                stats = ln2_pool.tile([128, 2, 6], F32, tag="st2")
                for g in range(2):
                    nc.vector.bn_stats(out=stats[:, g, :], in_=x1g[:, g, :])
                nc.vector.bn_aggr(out=mv_all[:, m, :], in_=stats[:])
            # batched rstd/nmu for all 4 blocks (one ACT round trip)
            rstd = ln2_pool.tile([128, 4], F32, tag="rstd2", bufs=1)
            nc.scalar.activation(
                out=rstd[:], in_=mv_all[:, :, 1],
                func=AF.Sqrt, bias=eps_t[:], scale=1.0,
            )
            nc.vector.reciprocal(out=rstd[:], in_=rstd[:])
            nmu = ln2_pool.tile([128, 4], F32, tag="nmu2", bufs=1)
            nc.vector.tensor_tensor(
                out=nmu[:], in0=mv_all[:, :, 0], in1=rstd[:],
                op=mybir.AluOpType.mult,
            )
            nc.vector.tensor_scalar(
                out=nmu[:], in0=nmu[:],
                scalar1=-1.0, scalar2=None, op0=mybir.AluOpType.mult,
            )
            for m in range(4):
                h2 = ln2_pool.tile([128, C], BF16, tag="h2")
                nc.scalar.activation(
                    out=h2[:], in_=x1_sb[:, m, :], func=AF.Identity,
                    bias=nmu[:, m:m + 1], scale=rstd[:, m:m + 1],
                )
                tp2 = tp2_ps.tile([128, 6, 128], BF16, tag="tp2")
                for c in range(6):
                    nc.tensor.transpose(tp2[:, c, :], h2[:, c * 128:(c + 1) * 128], id_b[:])
                nc.vector.tensor_copy(
                    h2T_sb[:, :, m * 128:(m + 1) * 128], tp2[:])
                # fold the proj bias into the residual now (off the critical
                # path); LN2 above already consumed the raw x1.
                nc.vector.tensor_add(
                    x1_sb[:, m, :], x1_sb[:, m, :], bpj_bc[:])
                if m == 0:
                    # pull the gelu table load off the critical path: emitted
                    # after the first h2 apply, it loads while the remaining
                    # h2/transposes run (Identity is a filler in every set)
                    dummy = ln2_pool.tile([128, 1], F32, tag="dummy", bufs=1)
                    nc.scalar.activation(
                        out=dummy[:], in_=eps_t[:], func=AF.Gelu_apprx_tanh,
                        bias=0.0, scale=1.0,
                    )

            s4.close()
            # ---- stage 5: fc + gelu (weights resident, zero DMA stalls) ----
            s5 = ExitStack()
            pf_ps = s5.enter_context(tc.tile_pool(name="pf_ps", bufs=4, space="PSUM"))
            for f in range(24):
                pf = pf_ps.tile([128, 512], F32, tag="pf")
                for c in range(6):
                    nc.tensor.matmul(
                        pf[:], wfc_t[:, c, f * 128:(f + 1) * 128],
                        h2T_sb[:, c, :],
                        start=(c == 0), stop=(c == 5),
                    )
                nc.scalar.activation(
                    out=gT_sb[:, f, :], in_=pf[:],
                    func=AF.Gelu_apprx_tanh,
                    bias=bfc_sb[:, f:f + 1], scale=1.0,
                )

            s5.close()
            # ---- stage 6: proj + residual + store, m-outer so each block's
            # epilogue overlaps the next block's matmuls ----
            s6 = ExitStack()
            pj_ps = s6.enter_context(tc.tile_pool(name="pj_ps", bufs=2, space="PSUM"))
            for m in range(4):
                pj = []
                for half in range(2):
                    pj_i = pj_ps.tile([128, 384], F32, tag=f"pj{half}",
                                      name=f"pj{m}_{half}")
                    pj.append(pj_i)
                for f in range(24):
                    for half in range(2):
                        nc.tensor.matmul(
                            pj[half][:],
                            gT_sb[:, f, m * 128:(m + 1) * 128],
                            wpj_t[:, f, half * 384:(half + 1) * 384],
                            start=(f == 0), stop=(f == 23),
                        )
                o_t = out_pool.tile([128, C], F32, tag="o")
                for half in range(2):
                    nc.vector.tensor_add(
                        o_t[:, half * 384:(half + 1) * 384],
                        pj[half][:],
                        x1_sb[:, m, half * 384:(half + 1) * 384],
                    )
                nc.sync.dma_start(out=ins["out"][m * 128:(m + 1) * 128, :], in_=o_t[:])
            s6.close()


# ---------------------------------------------------------------------------
# Runner
# ---------------------------------------------------------------------------
def _make_runner(nc):
    partition_name = nc.partition_id_tensor.name if nc.partition_id_tensor else None
    in_names, out_names, out_avals, zero_outs = [], [], [], []
    for alloc in nc.m.functions[0].allocations:
        if not isinstance(alloc, mybir.MemoryLocationSet):
            continue
        name = alloc.memorylocations[0].name
        if alloc.kind == "ExternalInput":
            if name != partition_name:
                in_names.append(name)
        elif alloc.kind == "ExternalOutput":
            out_names.append(name)
            shape = tuple(alloc.tensor_shape)
            dtype = mybir.dt.np(alloc.dtype)
            out_avals.append(jax.core.ShapedArray(shape, dtype))
            zero_outs.append(np.zeros(shape, dtype))
    n_params = len(in_names)
    all_names = list(in_names) + list(out_names)
    if partition_name is not None:
        all_names.append(partition_name)

    def _body(*args):
        operands = list(args)
        if partition_name is not None:
            operands.append(partition_id_tensor())
        outs = _bass_exec_p.bind(
            *operands,
            out_avals=tuple(out_avals),
            in_names=tuple(all_names),
            out_names=tuple(out_names),
            lowering_input_output_aliases=(),
            sim_require_finite=True,
            sim_require_nnan=True,
            nc=nc,
        )
        return tuple(outs)

    donate = tuple(range(n_params, n_params + len(out_names)))
    jitted = jax.jit(_body, donate_argnums=donate, keep_unused=True)
    return jitted, in_names, out_names, zero_outs


@functools.lru_cache(maxsize=None)
def _get_runners():
    install_neuronx_cc_hook()
    runners = []
    for r in range(4):
        nc = build_rank_program(r)
        runners.append(_make_runner(nc))
    return runners


def _prep_core_inputs(x, ln1_w, ln1_b, c_attn_w, c_attn_b, c_proj_w, c_proj_b,
                      ln2_w, ln2_b, fc_w, fc_b, proj_w, proj_b):
    """Fold LN affines into weights; split qkv; cast weights to bf16."""
    f32 = np.float32
    bf16 = ml_dtypes.bfloat16
    wqkv = (ln1_w[:, None] * c_attn_w).astype(f32)
    bqkv = (c_attn_b + ln1_b @ c_attn_w).astype(f32)
    scale = f32(1.0 / np.sqrt(HD))
    shared = {
        "wq": np.ascontiguousarray((wqkv[:, 0:C] * scale).astype(bf16)),
        "wk": np.ascontiguousarray(wqkv[:, C:2 * C].astype(bf16)),
        "wv": np.ascontiguousarray(wqkv[:, 2 * C:3 * C].astype(bf16)),
        "bq": np.ascontiguousarray(bqkv[0:C] * scale),
        "bv": np.ascontiguousarray(bqkv[2 * C:3 * C]),
        "wcp": np.ascontiguousarray(c_proj_w.astype(bf16)),
        "bcp": np.ascontiguousarray(c_proj_b.astype(f32)),
        "wfc": np.ascontiguousarray((ln2_w[:, None] * fc_w).astype(bf16)),
        "bfc": np.ascontiguousarray((fc_b + ln2_b @ fc_w).astype(f32)),
        "wpj": np.ascontiguousarray(proj_w.astype(bf16)),
        "bpj": np.ascontiguousarray(proj_b.astype(f32)),
    }
    return shared


def _dispatch_all(inputs):
    """Dispatch the 8 per-core executions asynchronously; return futures."""
    runners = _get_runners()
    devices = jax.devices()
    shared = _prep_core_inputs(**{k: np.asarray(v) for k, v in inputs.items()})
    x = np.asarray(inputs["x"], dtype=np.float32)
    futs = []
    for c in range(8):
        b, r = c // 4, c % 4
        jitted, in_names, out_names, zero_outs = runners[r]
        dev = devices[c]
        per_core = dict(shared)
        per_core["x"] = np.ascontiguousarray(x[b])
        args = [jax.device_put(per_core[n], dev) for n in in_names]
        args += [jax.device_put(z, dev) for z in zero_outs]
        futs.append((c, out_names, jitted(*args)))
    return futs


def kernel(**inputs) -> np.ndarray:
    futs = _dispatch_all(inputs)
    out = np.empty((B, T, C), dtype=np.float32)
    for c, out_names, fut in futs:
        b, r = c // 4, c % 4
        res = np.asarray(fut[out_names.index("out")])
        out[b, 256 * r:256 * r + 256] = res[0:256]
        out[b, 256 * (7 - r):256 * (7 - r) + 256] = res[256:512]
    return out
